# revision 1
# baseline (speedup 1.0000x reference)
"""Trainium2 Bass kernel for nn_BreakthroughSNN (spiking SSM LM).

Strategy (8 NeuronCores, SPMD single NEFF):
  - Data-parallel SSM: 2048 tokens (B*S) sharded 256/core. Per core, the
    4-layer x 20-step LIF recurrence runs with persistent membrane
    potentials held in PSUM (PE accumulates state/output updates, DVE
    applies the leak*reset in place, ACT computes sign(v-thr), GPSIMD
    emits spikes).
  - All SSM matmuls are fp32r hi/lo pairs (host-split so the device's
    fp32r rounding is exact) -> full fp32-grade precision at 1 cyc/row.
  - Temporal encoding via host-precomputed exact fp32 sigmoid-boundary
    thresholds (no device sigmoid -> bit-exact one-hot vs fp32 ref).
  - Vocab-sharded output projection: time-integrated rates are
    AllGathered (bf16, tiny) so each core computes all 2048 tokens x its
    4000-vocab shard; Wp streamed as bf16 (post-chaos linear op).
"""

import numpy as np
import ml_dtypes
from contextlib import ExitStack

import concourse.bass as bass
import concourse.mybir as mybir
import concourse.tile as tile
from concourse import bacc
from concourse.bass_utils import run_bass_kernel_spmd
from concourse.masks import make_identity

F32 = mybir.dt.float32
F32R = mybir.dt.float32r
BF16 = mybir.dt.bfloat16
I32 = mybir.dt.int32
OP = mybir.AluOpType
ACTF = mybir.ActivationFunctionType

NCORES = 8
TOKPC = 256          # tokens per core
BATCH, SEQ = 4, 512
DM, DS = 512, 128
T, L = 20, 4
VOC = 32000
VSH = VOC // NCORES  # 4000 vocab per core
NV = 500             # vocab cols per proj tile (one PSUM bank; 8 tiles per core)
KC = DM // 128       # 4 feature chunks


def _hilo(x):
    x = np.ascontiguousarray(x, dtype=np.float32)
    u = x.view(np.uint32)
    hi = (u & np.uint32(0xFFFFF000)).view(np.float32).copy()  # keep 11 mantissa bits
    lo = (x - hi).astype(np.float32)
    return hi, lo


def _f2key(x):
    u = int(np.array(x, dtype=np.float32).view(np.uint32))
    return (u ^ 0x80000000) if u < 0x80000000 else (0xFFFFFFFF - u)


def _key2f(k):
    u = (k ^ 0x80000000) if k >= 0x80000000 else (0xFFFFFFFF - k)
    return np.array([u], dtype=np.uint32).view(np.float32)[0]


def _g32(x):
    # replicate reference fp32 pipeline: floor happens on this value
    x = np.float32(x)
    s = np.float32(1.0) / (np.float32(1.0) + np.float32(np.exp(np.float32(-x))))
    return np.float32(s * np.float32(19.0))


def _thresholds():
    """T_k = smallest fp32 x with g32(x) >= k, k=1..19 (g32 monotone)."""
    ts = []
    for k in range(1, 20):
        lo_k = _f2key(np.float32(-30.0))
        hi_k = _f2key(np.float32(30.0))
        assert _g32(_key2f(hi_k)) >= k and _g32(_key2f(lo_k)) < k
        while hi_k - lo_k > 1:
            mid = (lo_k + hi_k) // 2
            if _g32(_key2f(mid)) >= k:
                hi_k = mid
            else:
                lo_k = mid
        ts.append(float(_key2f(hi_k)))
    return ts


def _build_nc():
    nc = bacc.Bacc("TRN2", target_bir_lowering=False, debug=False, num_devices=NCORES)

    ids_d = nc.dram_tensor("ids", [2, 128, 1], I32, kind="ExternalInput")
    emb_d = nc.dram_tensor("emb", [VOC, DM], F32, kind="ExternalInput")
    at_hi_d = nc.dram_tensor("at_hi", [L, 128, 128], F32, kind="ExternalInput")
    at_lo_d = nc.dram_tensor("at_lo", [L, 128, 128], F32, kind="ExternalInput")
    bt_hi_d = nc.dram_tensor("bt_hi", [L, 128, KC, 128], F32, kind="ExternalInput")
    bt_lo_d = nc.dram_tensor("bt_lo", [L, 128, KC, 128], F32, kind="ExternalInput")
    ct_hi_d = nc.dram_tensor("ct_hi", [L, 128, KC, 128], F32, kind="ExternalInput")
    ct_lo_d = nc.dram_tensor("ct_lo", [L, 128, KC, 128], F32, kind="ExternalInput")
    dc_hi_d = nc.dram_tensor("dc_hi", [L, 128, KC], F32, kind="ExternalInput")
    dc_lo_d = nc.dram_tensor("dc_lo", [L, 128, KC], F32, kind="ExternalInput")
    wpt_d = nc.dram_tensor("wpt", [DM, VSH], BF16, kind="ExternalInput")
    bias_d = nc.dram_tensor("bias", [1, VSH], F32, kind="ExternalInput")
    out_d = nc.dram_tensor("out", [TOKPC * NCORES, VSH], F32, kind="ExternalOutput")

    THR = _thresholds()

    with tile.TileContext(nc) as tc, ExitStack() as ctx:
        const = ctx.enter_context(tc.tile_pool(name="const", bufs=1))
        ident = const.tile([128, 128], F32)
        make_identity(nc, ident[:])
        ident_r = const.tile([128, 128], F32R)
        nc.vector.tensor_copy(ident_r[:], ident[:])
        neg2 = const.tile([128, 1], F32)
        nc.vector.memset(neg2[:], -2.0)

        xb_pool = ctx.enter_context(tc.tile_pool(name="xb", bufs=1))
        xb = xb_pool.tile([128, T * KC * 256], F32R)
        tip = ctx.enter_context(tc.tile_pool(name="ti", bufs=1))
        tibf = tip.tile([128, KC * 256], BF16, tag="tibf")

        # ---------------- encode: gather + transpose + thresholds ----------
        with tc.tile_pool(name="enc", bufs=2) as enc, \
             tc.tile_pool(name="encp", bufs=2, space="PSUM") as encps, \
             tc.tile_pool(name="emb4", bufs=1) as emb4:
            ids_s = enc.tile([128, 2], I32, tag="ids")
            for g in range(2):
                nc.sync.dma_start(ids_s[:, g:g + 1], ids_d[g, :, :])
            EMB = [emb4.tile([128, TOKPC], F32, tag=f"emb{k}", name=f"EMB{k}") for k in range(KC)]
            IDX = [emb4.tile([128, TOKPC], F32, tag=f"idx{k}", name=f"IDX{k}") for k in range(KC)]
            for g in range(2):
                eg = enc.tile([128, DM], F32, tag="eg")
                nc.gpsimd.indirect_dma_start(
                    out=eg[:], out_offset=None,
                    in_=emb_d[:, :],
                    in_offset=bass.IndirectOffsetOnAxis(ap=ids_s[:, g:g + 1], axis=0),
                )
                for k in range(KC):
                    pt = encps.tile([128, 128], F32, tag="pt")
                    nc.tensor.transpose(pt[:], eg[:, k * 128:(k + 1) * 128], ident[:])
                    nc.scalar.copy(EMB[k][:, g * 128:(g + 1) * 128], pt[:])
            ge_t = [emb4.tile([128, TOKPC], F32, tag=f"ge{k}", name=f"ge{k}")
                    for k in range(2)]
            for k in range(KC):
                if k < 2:
                    # DVE chain: fused compare-accumulate
                    nc.vector.memset(IDX[k][:], 0.0)
                    for tj in THR:
                        nc.vector.scalar_tensor_tensor(IDX[k][:], EMB[k][:],
                                                       float(tj), IDX[k][:],
                                                       OP.is_ge, OP.add)
                else:
                    # GPSIMD chain: compare then accumulate (no stt on Pool)
                    g = ge_t[k - 2]
                    nc.gpsimd.tensor_scalar(IDX[k][:], EMB[k][:], float(THR[0]),
                                            None, OP.is_ge)
                    for tj in THR[1:]:
                        nc.gpsimd.tensor_scalar(g[:], EMB[k][:], float(tj),
                                                None, OP.is_ge)
                        nc.gpsimd.tensor_tensor(IDX[k][:], IDX[k][:], g[:], OP.add)
            # one-hot spikes into X buffer (values {0,1}, fp32r-exact)
            for t in range(T):
                for k in range(KC):
                    nc.vector.tensor_scalar(
                        xb[:, (t * KC + k) * 256:(t * KC + k) * 256 + 256],
                        IDX[k][:], float(t), None, OP.is_equal)

        # ---------------- SSM layers ---------------------------------------
        with tc.tile_pool(name="ssmp", bufs=1, space="PSUM") as ssmps, \
             tc.tile_pool(name="par", bufs=2) as par, \
             tc.tile_pool(name="stg", bufs=2) as stg, \
             tc.tile_pool(name="lif", bufs=3) as lif:
            v1ps = ssmps.tile([128, TOKPC], F32, tag="v1")
            # v2 as two (128,512) tiles: pair j holds feature chunks 2j, 2j+1
            # side by side in the free dim (each tile = exactly one PSUM bank)
            v2pr = [ssmps.tile([128, 2 * TOKPC], F32, tag=f"v2p{j}", name=f"v2pr{j}")
                    for j in range(2)]
            tips = ssmps.tile([128, KC * TOKPC], F32, tag="tips")

            Hprev = None
            for layer in range(L):
                # -- param prep (hi/lo fp32r tiles; host pre-rounded) --
                def load_rounded(dram_ap, shape, tag):
                    st = stg.tile(list(shape), F32, tag="stage")
                    nc.sync.dma_start(st[:], dram_ap)
                    pt_ = par.tile(list(shape), F32R, tag=tag, name=f"par_{tag}")
                    nc.vector.tensor_copy(pt_[:], st[:])
                    return pt_

                ah = load_rounded(at_hi_d[layer, :, :], (128, 128), "ah")
                al = load_rounded(at_lo_d[layer, :, :], (128, 128), "al")
                bh = load_rounded(bt_hi_d[layer, :, :, :], (128, KC, 128), "bh")
                bl = load_rounded(bt_lo_d[layer, :, :, :], (128, KC, 128), "bl")
                ch = load_rounded(ct_hi_d[layer, :, :, :], (128, KC, 128), "ch")
                cl = load_rounded(ct_lo_d[layer, :, :, :], (128, KC, 128), "cl")
                dch = stg.tile([128, KC], F32, tag="dch")
                nc.sync.dma_start(dch[:], dc_hi_d[layer, :, :])
                dcl = stg.tile([128, KC], F32, tag="dcl")
                nc.sync.dma_start(dcl[:], dc_lo_d[layer, :, :])
                ddh, ddl = [], []
                for k in range(KC):
                    dt_ = par.tile([128, 128], F32R, tag=f"ddh{k}", name=f"ddh{k}")
                    nc.vector.tensor_scalar(dt_[:], ident[:], dch[:, k:k + 1], None, OP.mult)
                    ddh.append(dt_)
                    dt_ = par.tile([128, 128], F32R, tag=f"ddl{k}", name=f"ddl{k}")
                    nc.vector.tensor_scalar(dt_[:], ident[:], dcl[:, k:k + 1], None, OP.mult)
                    ddl.append(dt_)

                def emit_mm2_lif2(t, H_t, xs_t, layer_):
                    # output update accumulation (v2, per chunk) + LIF2
                    for k in range(KC):
                        vsl = v2pr[k // 2][:, (k % 2) * TOKPC:(k % 2 + 1) * TOKPC]
                        mm2 = [(ch[:, k, :], H_t[:]), (cl[:, k, :], H_t[:]),
                               (ddh[k][:], xs_t[k]), (ddl[k][:], xs_t[k])]
                        for i, (lhsT, rhs) in enumerate(mm2):
                            # start=True clears the WHOLE bank -> only the
                            # first MM into each bank per layer may set it;
                            # the pair sibling begins on has_written=0.
                            nc.tensor.matmul(vsl, lhsT, rhs,
                                             start=(t == 0 and i == 0 and k % 2 == 0),
                                             stop=(i == len(mm2) - 1),
                                             skip_group_check=True)
                    for j in range(2):
                        xsl = xb[:, (t * KC + 2 * j) * 256:(t * KC + 2 * j) * 256 + 512]
                        m2 = lif.tile([128, 2 * TOKPC], F32, tag=f"m2_{j}",
                                      name=f"m2_{j}")
                        if j == 0:
                            # DVE straight from PSUM; spike via GPSIMD from m2
                            nc.vector.tensor_scalar(m2[:], v2pr[j][:], 2.0, 0.5,
                                                    OP.is_lt, OP.mult)
                            nc.gpsimd.tensor_scalar(xsl, m2[:], -2.0, 1.0,
                                                    OP.mult, OP.add)
                        else:
                            sg2 = lif.tile([128, 2 * TOKPC], F32, tag="sg2",
                                           name="sg2")
                            nc.scalar.activation(sg2[:], v2pr[j][:], ACTF.Sign,
                                                 bias=neg2[:], scale=1.0)
                            nc.gpsimd.tensor_scalar(m2[:], sg2[:], -0.25, 0.25,
                                                    OP.mult, OP.add)
                            nc.gpsimd.tensor_scalar(xsl, sg2[:], 0.5, 0.5,
                                                    OP.mult, OP.add)
                        nc.vector.tensor_tensor(v2pr[j][:], v2pr[j][:], m2[:], OP.mult)
                    if layer_ == L - 1:
                        # time-integration on the PE: tips += I @ X[t]
                        # (tips spans 2 banks: slices 0,1 / 2,3 -> one
                        # start=True per bank, at k==0 and k==2)
                        for k in range(KC):
                            nc.tensor.matmul(
                                tips[:, k * TOKPC:(k + 1) * TOKPC],
                                ident_r[:], xs_t[k],
                                start=(t == 0 and k % 2 == 0),
                                stop=(t == T - 1),
                                skip_group_check=True)

                prev = None  # (t, H, xs) pending MM2+LIF2 (1-step software skew)
                for t in range(T):
                    xs = [xb[:, (t * KC + k) * 256:(t * KC + k) * 256 + 256]
                          for k in range(KC)]
                    # ---- state update accumulation (v1) ----
                    mm1 = []
                    if t > 0:
                        mm1 += [(ah[:], Hprev[:]), (al[:], Hprev[:])]
                    for k in range(KC):
                        mm1 += [(bh[:, k, :], xs[k]), (bl[:, k, :], xs[k])]
                    for i, (lhsT, rhs) in enumerate(mm1):
                        nc.tensor.matmul(v1ps[:], lhsT, rhs,
                                         start=(t == 0 and i == 0),
                                         stop=(i == len(mm1) - 1),
                                         skip_group_check=True)
                    # ---- LIF1: spike H straight from PSUM (critical path),
                    #      then leak mask m1 derived from H off-path ----
                    H = lif.tile([128, TOKPC], F32R, tag="H", bufs=3)
                    nc.vector.tensor_scalar(H[:], v1ps[:], 2.0, None, OP.is_ge)
                    m1 = lif.tile([128, TOKPC], F32, tag="m1")
                    nc.vector.tensor_scalar(m1[:], H[:].bitcast(F32), -0.5, 0.5,
                                            OP.mult, OP.add)
                    nc.vector.tensor_tensor(v1ps[:], v1ps[:], m1[:], OP.mult)
                    # ---- previous step's output-side work (keeps PE fed) ----
                    if prev is not None:
                        emit_mm2_lif2(*prev, layer)
                    prev = (t, H, xs)
                    Hprev = H
                emit_mm2_lif2(*prev, layer)

            # time-integrated rates -> bf16 (tips psum holds sum over T)
            for k in range(KC):
                nc.vector.tensor_scalar(tibf[:, k * 256:(k + 1) * 256],
                                        tips[:, k * TOKPC:(k + 1) * TOKPC],
                                        1.0 / T, None, OP.mult)

        # ---------------- allgather + projection ----------------------------
        with tc.tile_pool(name="agd", bufs=1, space="DRAM") as agd:
            agi = agd.tile([128, KC * 256], BF16)
            nc.sync.dma_start(agi[:], tibf[:])
            ago = agd.tile([NCORES * 128, KC * 256], BF16)
            nc.gpsimd.collective_compute(
                "AllGather", OP.bypass,
                replica_groups=[list(range(NCORES))],
                ins=[agi[:].opt()], outs=[ago[:].opt()],
            )
            tiall = tip.tile([128, NCORES, KC * 256], BF16, tag="tiall")
            nc.sync.dma_start(
                tiall[:],
                ago[:].rearrange("(n p) x -> p n x", n=NCORES, p=128))

            # ---------------- vocab-sharded projection ---------------------
            with tc.tile_pool(name="prj", bufs=2) as prj, \
                 tc.tile_pool(name="prjp", bufs=4, space="PSUM") as prjps, \
                 tc.tile_pool(name="osb", bufs=4) as osbp:
                mchunks = TOKPC * NCORES // 128
                for nv in range(VSH // NV):
                    bias_bc = prj.tile([128, NV], F32, tag="bias")
                    bap = bias_d[0:1, nv * NV:(nv + 1) * NV]
                    bsrc = bass.AP(tensor=bap.tensor, offset=bap.offset,
                                   ap=[[0, 128], [1, NV]])
                    nc.sync.dma_start(bias_bc[:], bsrc)
                    wts = []
                    for k in range(KC):
                        wt = prj.tile([128, NV], BF16, tag=f"wt{k}", name=f"wt{k}")
                        nc.sync.dma_start(wt[:], wpt_d[k * 128:(k + 1) * 128,
                                                       nv * NV:(nv + 1) * NV])
                        wts.append(wt)
                    for m in range(mchunks):
                        c, half = divmod(m, 2)
                        po = prjps.tile([128, NV], F32, tag="po")
                        for k in range(KC):
                            lh = tiall[:, c, k * 256 + half * 128:
                                       k * 256 + half * 128 + 128]
                            nc.tensor.matmul(po[:], lh, wts[k][:],
                                             start=(k == 0), stop=(k == KC - 1),
                                             skip_group_check=True)
                        osb = osbp.tile([128, NV], F32, tag="osb")
                        nc.vector.tensor_tensor(osb[:], po[:], bias_bc[:], OP.add)
                        nc.sync.dma_start(out_d[m * 128:(m + 1) * 128,
                                                nv * NV:(nv + 1) * NV], osb[:])

    nc.compile()
    return nc


_NC_CACHE = {}
_last_in_maps = None


def _get_nc():
    if "nc" not in _NC_CACHE:
        _NC_CACHE["nc"] = _build_nc()
    return _NC_CACHE["nc"]


def kernel(input_ids, emb_table, A, B, C, D, Wp, bp):
    input_ids = np.asarray(input_ids)
    emb_table = np.ascontiguousarray(np.asarray(emb_table), dtype=np.float32)
    A = np.asarray(A, dtype=np.float32)
    B = np.asarray(B, dtype=np.float32)
    C = np.asarray(C, dtype=np.float32)
    D = np.asarray(D, dtype=np.float32)
    Wp = np.asarray(Wp, dtype=np.float32)
    bp = np.asarray(bp, dtype=np.float32)

    ids_flat = input_ids.reshape(-1).astype(np.int32)          # (2048,)

    at = np.ascontiguousarray(A.transpose(0, 2, 1))            # (L,128,128)
    at_hi, at_lo = _hilo(at)
    bt = np.ascontiguousarray(
        B.transpose(2, 0, 1).reshape(KC, 128, L, DS).transpose(2, 1, 0, 3))
    # bt[l,p,k,m] = B[l, m, k*128+p]
    bt_hi, bt_lo = _hilo(bt)
    ct = np.ascontiguousarray(C.transpose(0, 2, 1).reshape(L, 128, KC, 128))
    # ct[l,p,mc,m] = C[l, mc*128+m, p]
    ct_hi, ct_lo = _hilo(ct)
    dc = np.ascontiguousarray(D.reshape(L, KC, 128).transpose(0, 2, 1))  # (L,128,KC)
    dc_hi, dc_lo = _hilo(dc)

    wpt = np.ascontiguousarray(Wp.T)                           # (512, 32000) f32
    wpt_bf = wpt.astype(ml_dtypes.bfloat16)

    nc = _get_nc()
    in_maps = []
    for c in range(NCORES):
        ids_c = ids_flat[c * TOKPC:(c + 1) * TOKPC].reshape(2, 128, 1)
        in_maps.append({
            "ids": np.ascontiguousarray(ids_c),
            "emb": emb_table,
            "at_hi": at_hi, "at_lo": at_lo,
            "bt_hi": bt_hi, "bt_lo": bt_lo,
            "ct_hi": ct_hi, "ct_lo": ct_lo,
            "dc_hi": dc_hi, "dc_lo": dc_lo,
            "wpt": np.ascontiguousarray(wpt_bf[:, c * VSH:(c + 1) * VSH]),
            "bias": np.ascontiguousarray(bp[c * VSH:(c + 1) * VSH]).reshape(1, VSH),
        })

    global _last_in_maps
    _last_in_maps = in_maps
    res = run_bass_kernel_spmd(nc, in_maps, core_ids=list(range(NCORES)))
    outs = [res.results[c]["out"] for c in range(NCORES)]
    full = np.concatenate(outs, axis=1)                        # (2048, 32000)
    return full.reshape(BATCH, SEQ, VOC).astype(np.float32)



# revision 9
# speedup vs baseline: 1.1650x; 1.1650x over previous
"""Trainium2 Bass kernel for nn_BreakthroughSNN (spiking SSM LM).

Strategy (8 NeuronCores, SPMD single NEFF, fully independent cores):
  - Data-parallel SSM: 2048 tokens (B*S) sharded 256/core. Per core, the
    4-layer x 20-step LIF recurrence runs with persistent membrane
    potentials held in PSUM.
  - All SSM matmuls are fp32r hi/lo pairs (host-split so the device's
    fp32r rounding is exact) -> full fp32-grade precision at 1 cyc/row.
  - Temporal encoding via host-precomputed exact fp32 sigmoid-boundary
    thresholds: per-chunk threshold-count index built with fused
    is_ge/add chains on DVE; the one-hot spike planes are materialized
    just-in-time inside layer 0's step loop (is_equal per step).
  - LIF elementwise work spread across ACT (Sign from PSUM, spike
    writes), GPSIMD (mask affine derivations in SBUF) and DVE (PSUM
    multiply updates) so no single engine gates the recurrence.
  - Projection: token-sharded - each core projects its OWN 256 tokens
    against the FULL 32000 vocab. Wp is streamed as bf16 (pre-scaled by
    1/T on host so the time-integrated spike counts stay integer-exact),
    double-buffered so DMA overlaps the matmuls. No collective at all.
    Output is written bf16 (post-chaos linear op) and upcast + bias on
    host.
"""

import numpy as np
import ml_dtypes
from contextlib import ExitStack

import concourse.bass as bass
import concourse.mybir as mybir
import concourse.tile as tile
from concourse import bacc
from concourse.bass_utils import run_bass_kernel_spmd
from concourse.masks import make_identity

F32 = mybir.dt.float32
F32R = mybir.dt.float32r
BF16 = mybir.dt.bfloat16
I32 = mybir.dt.int32
OP = mybir.AluOpType
ACTF = mybir.ActivationFunctionType

NCORES = 8
TOKPC = 256          # tokens per core
BATCH, SEQ = 4, 512
DM, DS = 512, 128
T, L = 20, 4
VOC = 32000
KC = DM // 128       # 4 feature chunks
NVW = 2000           # vocab cols per proj weight tile (4 psum banks of 500)
NVG = VOC // NVW     # 16 vocab groups
NV = 500             # one PSUM bank of fp32


def _hilo(x):
    x = np.ascontiguousarray(x, dtype=np.float32)
    u = x.view(np.uint32)
    hi = (u & np.uint32(0xFFFFF000)).view(np.float32).copy()  # keep 11 mantissa bits
    lo = (x - hi).astype(np.float32)
    return hi, lo


def _f2key(x):
    u = int(np.array(x, dtype=np.float32).view(np.uint32))
    return (u ^ 0x80000000) if u < 0x80000000 else (0xFFFFFFFF - u)


def _key2f(k):
    u = (k ^ 0x80000000) if k >= 0x80000000 else (0xFFFFFFFF - k)
    return np.array([u], dtype=np.uint32).view(np.float32)[0]


def _g32(x):
    # replicate reference fp32 pipeline: floor happens on this value
    x = np.float32(x)
    s = np.float32(1.0) / (np.float32(1.0) + np.float32(np.exp(np.float32(-x))))
    return np.float32(s * np.float32(19.0))


def _thresholds():
    """T_k = smallest fp32 x with g32(x) >= k, k=1..19 (g32 monotone)."""
    ts = []
    for k in range(1, 20):
        lo_k = _f2key(np.float32(-30.0))
        hi_k = _f2key(np.float32(30.0))
        assert _g32(_key2f(hi_k)) >= k and _g32(_key2f(lo_k)) < k
        while hi_k - lo_k > 1:
            mid = (lo_k + hi_k) // 2
            if _g32(_key2f(mid)) >= k:
                hi_k = mid
            else:
                lo_k = mid
        ts.append(float(_key2f(hi_k)))
    return ts


def _build_nc():
    nc = bacc.Bacc("TRN2", target_bir_lowering=False, debug=False, num_devices=NCORES)

    ids_d = nc.dram_tensor("ids", [2, 128, 1], I32, kind="ExternalInput")
    emb_d = nc.dram_tensor("emb", [VOC, DM], F32, kind="ExternalInput")
    at_hi_d = nc.dram_tensor("at_hi", [L, 128, 128], F32R, kind="ExternalInput")
    at_lo_d = nc.dram_tensor("at_lo", [L, 128, 128], F32R, kind="ExternalInput")
    bt_hi_d = nc.dram_tensor("bt_hi", [L, 128, KC, 128], F32R, kind="ExternalInput")
    bt_lo_d = nc.dram_tensor("bt_lo", [L, 128, KC, 128], F32R, kind="ExternalInput")
    ct_hi_d = nc.dram_tensor("ct_hi", [L, 128, KC, 128], F32R, kind="ExternalInput")
    ct_lo_d = nc.dram_tensor("ct_lo", [L, 128, KC, 128], F32R, kind="ExternalInput")
    dd_hi_d = nc.dram_tensor("dd_hi", [L, 128, KC, 128], F32R, kind="ExternalInput")
    dd_lo_d = nc.dram_tensor("dd_lo", [L, 128, KC, 128], F32R, kind="ExternalInput")
    wpt_d = nc.dram_tensor("wpt", [DM, VOC], BF16, kind="ExternalInput")
    out_d = nc.dram_tensor("out", [TOKPC, VOC], BF16, kind="ExternalOutput")

    THR = _thresholds()

    with tile.TileContext(nc) as tc, ExitStack() as ctx:
        const = ctx.enter_context(tc.tile_pool(name="const", bufs=1))
        ident = const.tile([128, 128], F32)
        make_identity(nc, ident[:])
        ident_r = const.tile([128, 128], F32R)
        nc.vector.tensor_copy(ident_r[:], ident[:])
        neg2 = const.tile([128, 1], F32)
        nc.vector.memset(neg2[:], -2.0)

        xb_pool = ctx.enter_context(tc.tile_pool(name="xb", bufs=1))
        xb = xb_pool.tile([128, T * KC * 256], F32R)
        tip = ctx.enter_context(tc.tile_pool(name="ti", bufs=1))
        ti_bf = tip.tile([128, KC * 256], BF16, tag="tibf")
        # projection weight pool created BEFORE the SSM pools so its SBUF
        # region doesn't overlap theirs -> the first weight-group DMAs can
        # prefetch during the SSM phase
        prw = ctx.enter_context(tc.tile_pool(name="prw", bufs=2))
        osbp = ctx.enter_context(tc.tile_pool(name="osb", bufs=3))

        # ---------------- encode: gather + transpose + idx chains ----------
        emb4 = ctx.enter_context(tc.tile_pool(name="emb4", bufs=1))
        IDXC = emb4.tile([128, KC * TOKPC], F32, tag="idxc")
        IDX = [IDXC[:, k * TOKPC:(k + 1) * TOKPC] for k in range(KC)]
        with tc.tile_pool(name="enc", bufs=2) as enc, \
             tc.tile_pool(name="encp", bufs=2, space="PSUM") as encps:
            ids_s = enc.tile([128, 2], I32, tag="ids")
            for g in range(2):
                nc.sync.dma_start(ids_s[:, g:g + 1], ids_d[g, :, :])
            EMB = [emb4.tile([128, TOKPC], F32, tag=f"emb{k}", name=f"EMB{k}")
                   for k in range(KC)]
            for g in range(2):
                eg = enc.tile([128, DM], F32, tag="eg")
                nc.gpsimd.indirect_dma_start(
                    out=eg[:], out_offset=None,
                    in_=emb_d[:, :],
                    in_offset=bass.IndirectOffsetOnAxis(ap=ids_s[:, g:g + 1], axis=0),
                )
                for k in range(KC):
                    pt = encps.tile([128, 128], F32, tag="pt")
                    nc.tensor.transpose(pt[:], eg[:, k * 128:(k + 1) * 128], ident[:])
                    nc.scalar.copy(EMB[k][:, g * 128:(g + 1) * 128], pt[:])
            # threshold-count index: IDX[k] = sum_j (EMB[k] >= T_j).
            # 4 independent chains interleaved on DVE so they pipeline.
            for k in range(KC):
                nc.vector.tensor_scalar(IDX[k], EMB[k][:], float(THR[0]),
                                        None, OP.is_ge)
            for tj in THR[1:]:
                for k in range(KC):
                    nc.vector.scalar_tensor_tensor(IDX[k], EMB[k][:],
                                                   float(tj), IDX[k],
                                                   OP.is_ge, OP.add)

        # ---------------- SSM layers ---------------------------------------
        with tc.tile_pool(name="ssmp", bufs=1, space="PSUM") as ssmps, \
             tc.tile_pool(name="par", bufs=2) as par, \
             tc.tile_pool(name="lif", bufs=3) as lif:
            v1ps = ssmps.tile([128, TOKPC], F32, tag="v1")
            # v2 as two (128,512) tiles: pair j holds feature chunks 2j, 2j+1
            v2pr = [ssmps.tile([128, 2 * TOKPC], F32, tag=f"v2p{j}", name=f"v2pr{j}")
                    for j in range(2)]
            tips = ssmps.tile([128, KC * TOKPC], F32, tag="tips")

            def onehot(t):
                # materialize spike plane xb[t] = (IDX == t) in one 1024-wide
                # op; alternate DVE/GPSIMD so neither engine gates layer 0
                eng = nc.vector if t % 2 == 0 else nc.gpsimd
                eng.tensor_scalar(
                    xb[:, t * KC * 256:(t + 1) * KC * 256],
                    IDXC[:], float(t), None, OP.is_equal)

            Hprev = None
            for layer in range(L):
                def loadp(dram_ap, shape, tag):
                    pt_ = par.tile(list(shape), F32R, tag=tag, name=f"par_{tag}")
                    nc.sync.dma_start(pt_[:], dram_ap)
                    return pt_

                ah = loadp(at_hi_d[layer, :, :], (128, 128), "ah")
                al = loadp(at_lo_d[layer, :, :], (128, 128), "al")
                bh = loadp(bt_hi_d[layer, :, :, :], (128, KC, 128), "bh")
                bl = loadp(bt_lo_d[layer, :, :, :], (128, KC, 128), "bl")
                ch = loadp(ct_hi_d[layer, :, :, :], (128, KC, 128), "ch")
                cl = loadp(ct_lo_d[layer, :, :, :], (128, KC, 128), "cl")
                dh = loadp(dd_hi_d[layer, :, :, :], (128, KC, 128), "dh")
                dl = loadp(dd_lo_d[layer, :, :, :], (128, KC, 128), "dl")

                def emit_mm2_lif2(t, H_t, xs_t, layer_):
                    # output update accumulation (v2, per chunk) + LIF2
                    for k in range(KC):
                        vsl = v2pr[k // 2][:, (k % 2) * TOKPC:(k % 2 + 1) * TOKPC]
                        mm2 = [(ch[:, k, :], H_t[:]), (cl[:, k, :], H_t[:]),
                               (dh[:, k, :], xs_t[k]), (dl[:, k, :], xs_t[k])]
                        for i, (lhsT, rhs) in enumerate(mm2):
                            # start=True clears the WHOLE bank -> only the
                            # first MM into each bank per layer may set it
                            nc.tensor.matmul(vsl, lhsT, rhs,
                                             start=(t == 0 and i == 0 and k % 2 == 0),
                                             stop=(i == len(mm2) - 1),
                                             skip_group_check=True)
                    # LIF2: sg on ACT (psum), mask on GPSIMD (sbuf),
                    # v2 *= mask on DVE (psum), spikes on DVE (one 1024 op,
                    # f32r out - the verifier requires DVE/GPSIMD producers
                    # for fp32r matmul operands)
                    m2c = lif.tile([128, 2 * 2 * TOKPC], F32, tag="m2c")
                    for j in range(2):
                        sg2 = lif.tile([128, 2 * TOKPC], F32, tag=f"sg2_{j}",
                                       name=f"sg2_{j}")
                        nc.scalar.activation(sg2[:], v2pr[j][:], ACTF.Sign,
                                             bias=neg2[:], scale=1.0)
                        nc.gpsimd.tensor_scalar(m2c[:, j * 512:(j + 1) * 512],
                                                sg2[:], -0.25, 0.25,
                                                OP.mult, OP.add)
                        nc.vector.tensor_tensor(v2pr[j][:], v2pr[j][:],
                                                m2c[:, j * 512:(j + 1) * 512],
                                                OP.mult)
                    xsl = xb[:, t * KC * 256:(t + 1) * KC * 256]
                    nc.vector.tensor_scalar(xsl, m2c[:], -2.0, 1.0,
                                            OP.mult, OP.add)
                    if layer_ == L - 1:
                        # time-integration on the PE: tips += I @ X[t]
                        # (reads xsl AFTER the spike overwrite above)
                        for k in range(KC):
                            nc.tensor.matmul(
                                tips[:, k * TOKPC:(k + 1) * TOKPC],
                                ident_r[:], xs_t[k],
                                start=(t == 0 and k % 2 == 0),
                                stop=(t == T - 1),
                                skip_group_check=True)

                if layer == 0:
                    onehot(0)

                prev = None  # (t, H, xs) pending MM2+LIF2 (1-step software skew)
                for t in range(T):
                    xs = [xb[:, (t * KC + k) * 256:(t * KC + k) * 256 + 256]
                          for k in range(KC)]
                    # ---- state update accumulation (v1) ----
                    mm1 = []
                    if t > 0:
                        mm1 += [(ah[:], Hprev[:]), (al[:], Hprev[:])]
                    for k in range(KC):
                        mm1 += [(bh[:, k, :], xs[k]), (bl[:, k, :], xs[k])]
                    for i, (lhsT, rhs) in enumerate(mm1):
                        nc.tensor.matmul(v1ps[:], lhsT, rhs,
                                         start=(t == 0 and i == 0),
                                         stop=(i == len(mm1) - 1),
                                         skip_group_check=True)
                    if layer == 0 and t + 1 < T:
                        onehot(t + 1)
                    # ---- LIF1: spike H straight from PSUM on DVE (critical
                    #      path: f32r matmul operands need DVE/GPSIMD
                    #      producers), m1 derived off-path on GPSIMD ----
                    H = lif.tile([128, TOKPC], F32R, tag="H", bufs=3)
                    nc.vector.tensor_scalar(H[:], v1ps[:], 2.0, None, OP.is_ge)
                    m1 = lif.tile([128, TOKPC], F32, tag="m1")
                    nc.gpsimd.tensor_scalar(m1[:], H[:].bitcast(F32), -0.5, 0.5,
                                            OP.mult, OP.add)
                    nc.vector.tensor_tensor(v1ps[:], v1ps[:], m1[:], OP.mult)
                    # ---- previous step's output-side work (keeps PE fed) ----
                    if prev is not None:
                        emit_mm2_lif2(*prev, layer)
                    prev = (t, H, xs)
                    Hprev = H
                emit_mm2_lif2(*prev, layer)

            # time-integrated spike counts -> bf16 (exact integers 0..20;
            # the 1/T scale is folded into Wp on the host)
            for j in range(2):
                nc.scalar.activation(ti_bf[:, j * 512:(j + 1) * 512],
                                     tips[:, j * 512:(j + 1) * 512],
                                     ACTF.Copy, bias=0.0, scale=1.0)

        # ---------------- projection: own 256 tokens x full vocab ----------
        with tc.tile_pool(name="prjp", bufs=2, space="PSUM") as prjps:
            for g in range(NVG):
                wts = []
                for k in range(KC):
                    wt = prw.tile([128, NVW], BF16, tag=f"wt{k}", name=f"wt{k}")
                    nc.sync.dma_start(wt[:], wpt_d[k * 128:(k + 1) * 128,
                                                   g * NVW:(g + 1) * NVW])
                    wts.append(wt)
                for m in range(TOKPC // 128):
                    pos = [prjps.tile([128, NV], F32, tag=f"po{nv}",
                                      name=f"po{nv}") for nv in range(NVW // NV)]
                    for k in range(KC):
                        lh = ti_bf[:, k * 256 + m * 128: k * 256 + m * 128 + 128]
                        for nv in range(NVW // NV):
                            nc.tensor.matmul(pos[nv][:], lh,
                                             wts[k][:, nv * NV:(nv + 1) * NV],
                                             start=(k == 0), stop=(k == KC - 1),
                                             skip_group_check=True)
                    osb = osbp.tile([128, NVW], BF16, tag="osb")
                    for nv in range(NVW // NV):
                        nc.scalar.activation(osb[:, nv * NV:(nv + 1) * NV],
                                             pos[nv][:], ACTF.Copy,
                                             bias=0.0, scale=1.0)
                    nc.sync.dma_start(out_d[m * 128:(m + 1) * 128,
                                            g * NVW:(g + 1) * NVW], osb[:])

    nc.compile()
    return nc


_NC_CACHE = {}
_last_in_maps = None


def _get_nc():
    if "nc" not in _NC_CACHE:
        _NC_CACHE["nc"] = _build_nc()
    return _NC_CACHE["nc"]


def kernel(input_ids, emb_table, A, B, C, D, Wp, bp):
    input_ids = np.asarray(input_ids)
    emb_table = np.ascontiguousarray(np.asarray(emb_table), dtype=np.float32)
    A = np.asarray(A, dtype=np.float32)
    B = np.asarray(B, dtype=np.float32)
    C = np.asarray(C, dtype=np.float32)
    D = np.asarray(D, dtype=np.float32)
    Wp = np.asarray(Wp, dtype=np.float32)
    bp = np.asarray(bp, dtype=np.float32)

    ids_flat = input_ids.reshape(-1).astype(np.int32)          # (2048,)

    at = np.ascontiguousarray(A.transpose(0, 2, 1))            # (L,128,128)
    at_hi, at_lo = _hilo(at)
    bt = np.ascontiguousarray(
        B.transpose(2, 0, 1).reshape(KC, 128, L, DS).transpose(2, 1, 0, 3))
    # bt[l,p,k,m] = B[l, m, k*128+p]
    bt_hi, bt_lo = _hilo(bt)
    ct = np.ascontiguousarray(C.transpose(0, 2, 1).reshape(L, 128, KC, 128))
    # ct[l,p,mc,m] = C[l, mc*128+m, p]
    ct_hi, ct_lo = _hilo(ct)
    dc = D.reshape(L, KC, 128)                                 # (L,KC,128)
    dc_hi, dc_lo = _hilo(dc)
    eye = np.eye(128, dtype=np.float32)
    # diag matrices for the D (elementwise) term, built on host
    dd_hi = np.ascontiguousarray(
        (eye[None, None, :, :] * dc_hi[:, :, None, :]).transpose(0, 2, 1, 3))
    dd_lo = np.ascontiguousarray(
        (eye[None, None, :, :] * dc_lo[:, :, None, :]).transpose(0, 2, 1, 3))
    # dd[l, p, k, m] = D[l, k*128+p] if p == m else 0

    wpt = np.ascontiguousarray(Wp.T) * np.float32(1.0 / T)     # (512, 32000) f32
    wpt_bf = wpt.astype(ml_dtypes.bfloat16)

    nc = _get_nc()
    in_maps = []
    for c in range(NCORES):
        ids_c = ids_flat[c * TOKPC:(c + 1) * TOKPC].reshape(2, 128, 1)
        in_maps.append({
            "ids": np.ascontiguousarray(ids_c),
            "emb": emb_table,
            "at_hi": at_hi, "at_lo": at_lo,
            "bt_hi": bt_hi, "bt_lo": bt_lo,
            "ct_hi": ct_hi, "ct_lo": ct_lo,
            "dd_hi": dd_hi, "dd_lo": dd_lo,
            "wpt": wpt_bf,
        })

    global _last_in_maps
    _last_in_maps = in_maps
    res = run_bass_kernel_spmd(nc, in_maps, core_ids=list(range(NCORES)))
    outs = [res.results[c]["out"].astype(np.float32) for c in range(NCORES)]
    full = np.concatenate(outs, axis=0)                        # (2048, 32000)
    full += bp[None, :]
    return full.reshape(BATCH, SEQ, VOC).astype(np.float32)


# revision 10
# speedup vs baseline: 1.4968x; 1.2847x over previous
"""Trainium2 Bass kernel for nn_BreakthroughSNN (spiking SSM LM).

Strategy (8 NeuronCores, SPMD single NEFF, fully independent cores):
  - Data-parallel SSM: 2048 tokens (B*S) sharded 256/core. Per core, the
    4-layer x 20-step LIF recurrence runs with persistent membrane
    potentials held in PSUM.
  - All SSM matmuls are fp32r hi/lo pairs (host-split so the device's
    fp32r rounding is exact) -> full fp32-grade precision at 1 cyc/row.
  - Temporal encoding via host-precomputed exact fp32 sigmoid-boundary
    thresholds: per-chunk threshold-count index built with fused
    is_ge/add chains on DVE; the one-hot spike planes are materialized
    just-in-time inside layer 0's step loop (is_equal per step).
  - LIF elementwise work spread across ACT (Sign from PSUM, spike
    writes), GPSIMD (mask affine derivations in SBUF) and DVE (PSUM
    multiply updates) so no single engine gates the recurrence.
  - Projection: token-sharded - each core projects its OWN 256 tokens
    against the FULL 32000 vocab. Wp is streamed as bf16 (pre-scaled by
    1/T on host so the time-integrated spike counts stay integer-exact),
    double-buffered so DMA overlaps the matmuls. No collective at all.
    Output is written bf16 (post-chaos linear op) and upcast + bias on
    host.
"""

import numpy as np
import ml_dtypes
from contextlib import ExitStack

import concourse.bass as bass
import concourse.mybir as mybir
import concourse.tile as tile
from concourse import bacc
from concourse.bass_utils import run_bass_kernel_spmd
from concourse.masks import make_identity

F32 = mybir.dt.float32
F32R = mybir.dt.float32r
BF16 = mybir.dt.bfloat16
I32 = mybir.dt.int32
OP = mybir.AluOpType
ACTF = mybir.ActivationFunctionType

NCORES = 8
TOKPC = 256          # tokens per core
BATCH, SEQ = 4, 512
DM, DS = 512, 128
T, L = 20, 4
VOC = 32000
KC = DM // 128       # 4 feature chunks
NVW = 2000           # vocab cols per proj weight tile (4 psum banks of 500)
NVG = VOC // NVW     # 16 vocab groups
NV = 500             # one PSUM bank of fp32


def _hilo(x):
    x = np.ascontiguousarray(x, dtype=np.float32)
    u = x.view(np.uint32)
    hi = (u & np.uint32(0xFFFFF000)).view(np.float32).copy()  # keep 11 mantissa bits
    lo = (x - hi).astype(np.float32)
    return hi, lo


def _f2key(x):
    u = int(np.array(x, dtype=np.float32).view(np.uint32))
    return (u ^ 0x80000000) if u < 0x80000000 else (0xFFFFFFFF - u)


def _key2f(k):
    u = (k ^ 0x80000000) if k >= 0x80000000 else (0xFFFFFFFF - k)
    return np.array([u], dtype=np.uint32).view(np.float32)[0]


def _g32(x):
    # replicate reference fp32 pipeline: floor happens on this value
    x = np.float32(x)
    s = np.float32(1.0) / (np.float32(1.0) + np.float32(np.exp(np.float32(-x))))
    return np.float32(s * np.float32(19.0))


def _thresholds():
    """T_k = smallest fp32 x with g32(x) >= k, k=1..19 (g32 monotone)."""
    ts = []
    for k in range(1, 20):
        lo_k = _f2key(np.float32(-30.0))
        hi_k = _f2key(np.float32(30.0))
        assert _g32(_key2f(hi_k)) >= k and _g32(_key2f(lo_k)) < k
        while hi_k - lo_k > 1:
            mid = (lo_k + hi_k) // 2
            if _g32(_key2f(mid)) >= k:
                hi_k = mid
            else:
                lo_k = mid
        ts.append(float(_key2f(hi_k)))
    return ts


def _build_nc():
    nc = bacc.Bacc("TRN2", target_bir_lowering=False, debug=False, num_devices=NCORES)

    ids_d = nc.dram_tensor("ids", [2, 128, 1], I32, kind="ExternalInput")
    emb_d = nc.dram_tensor("emb", [VOC, DM], F32, kind="ExternalInput")
    at_hi_d = nc.dram_tensor("at_hi", [L, 128, 128], F32R, kind="ExternalInput")
    at_lo_d = nc.dram_tensor("at_lo", [L, 128, 128], F32R, kind="ExternalInput")
    bt_hi_d = nc.dram_tensor("bt_hi", [L, 128, KC, 128], F32R, kind="ExternalInput")
    bt_lo_d = nc.dram_tensor("bt_lo", [L, 128, KC, 128], F32R, kind="ExternalInput")
    ct_hi_d = nc.dram_tensor("ct_hi", [L, 128, KC, 128], F32R, kind="ExternalInput")
    ct_lo_d = nc.dram_tensor("ct_lo", [L, 128, KC, 128], F32R, kind="ExternalInput")
    dd_hi_d = nc.dram_tensor("dd_hi", [L, 128, KC, 128], F32R, kind="ExternalInput")
    dd_lo_d = nc.dram_tensor("dd_lo", [L, 128, KC, 128], F32R, kind="ExternalInput")
    wpt_d = nc.dram_tensor("wpt", [DM, VOC], BF16, kind="ExternalInput")
    out_d = nc.dram_tensor("out", [TOKPC, VOC], BF16, kind="ExternalOutput")

    THR = _thresholds()

    with tile.TileContext(nc) as tc, ExitStack() as ctx:
        const = ctx.enter_context(tc.tile_pool(name="const", bufs=1))
        ident = const.tile([128, 128], F32)
        make_identity(nc, ident[:])
        ident_r = const.tile([128, 128], F32R)
        nc.vector.tensor_copy(ident_r[:], ident[:])
        neg2 = const.tile([128, 1], F32)
        nc.vector.memset(neg2[:], -2.0)

        xb_pool = ctx.enter_context(tc.tile_pool(name="xb", bufs=1))
        xb = xb_pool.tile([128, T * KC * 256], F32R)
        tip = ctx.enter_context(tc.tile_pool(name="ti", bufs=1))
        ti_bf = tip.tile([128, KC * 256], BF16, tag="tibf")
        # projection weight pool created BEFORE the SSM pools so its SBUF
        # region doesn't overlap theirs -> the first weight-group DMAs can
        # prefetch during the SSM phase
        prw = ctx.enter_context(tc.tile_pool(name="prw", bufs=2))
        osbp = ctx.enter_context(tc.tile_pool(name="osb", bufs=3))

        # ---------------- encode: gather + transpose + sign one-hot --------
        # Compare ALU ops are pathologically slow on DVE/GPSIMD; instead
        # build step functions SG_t = Sign(EMB - T_t) in {-1,1} on ACT
        # (IEEE subtract preserves the >= boundary exactly) and difference
        # them with fast DVE subtracts: xb[t] = SG_t - SG_{t+1} in {0,2}.
        # The x2 scale is compensated by halving layer 0's B and D on host.
        emb4 = ctx.enter_context(tc.tile_pool(name="emb4", bufs=1))
        EMBC = emb4.tile([128, KC * TOKPC], F32, tag="embc")
        thr_b = []
        for j, tj in enumerate(THR):
            bt_ = const.tile([128, 1], F32, tag=f"thr{j}", name=f"thr{j}")
            nc.vector.memset(bt_[:], -float(tj))
            thr_b.append(bt_)
        with tc.tile_pool(name="enc", bufs=2) as enc, \
             tc.tile_pool(name="encp", bufs=2, space="PSUM") as encps, \
             tc.tile_pool(name="sg", bufs=3) as sgp:
            ids_s = enc.tile([128, 2], I32, tag="ids")
            for g in range(2):
                nc.sync.dma_start(ids_s[:, g:g + 1], ids_d[g, :, :])
            for g in range(2):
                eg = enc.tile([128, DM], F32, tag="eg")
                nc.gpsimd.indirect_dma_start(
                    out=eg[:], out_offset=None,
                    in_=emb_d[:, :],
                    in_offset=bass.IndirectOffsetOnAxis(ap=ids_s[:, g:g + 1], axis=0),
                )
                for k in range(KC):
                    pt = encps.tile([128, 128], F32, tag="pt")
                    nc.tensor.transpose(pt[:], eg[:, k * 128:(k + 1) * 128], ident[:])
                    nc.scalar.copy(EMBC[:, k * 256 + g * 128:
                                        k * 256 + g * 128 + 128], pt[:])
            sg_prev = None
            for t in range(T - 1, 0, -1):
                sg_t = sgp.tile([128, KC * TOKPC], F32, tag="sg")
                nc.scalar.activation(sg_t[:], EMBC[:], ACTF.Sign,
                                     bias=thr_b[t - 1][:], scale=1.0)
                xsl = xb[:, t * KC * 256:(t + 1) * KC * 256]
                if t == T - 1:
                    nc.vector.tensor_scalar(xsl, sg_t[:], 1.0, None, OP.add)
                else:
                    nc.vector.tensor_tensor(xsl, sg_t[:], sg_prev[:],
                                            OP.subtract)
                sg_prev = sg_t
            nc.vector.tensor_scalar(xb[:, 0:KC * 256], sg_prev[:], -1.0, 1.0,
                                    OP.mult, OP.add)

        # ---------------- SSM layers ---------------------------------------
        with tc.tile_pool(name="ssmp", bufs=1, space="PSUM") as ssmps, \
             tc.tile_pool(name="par", bufs=2) as par, \
             tc.tile_pool(name="lif", bufs=3) as lif:
            v1ps = ssmps.tile([128, TOKPC], F32, tag="v1")
            # v2 as two (128,512) tiles: pair j holds feature chunks 2j, 2j+1
            v2pr = [ssmps.tile([128, 2 * TOKPC], F32, tag=f"v2p{j}", name=f"v2pr{j}")
                    for j in range(2)]
            tips = ssmps.tile([128, KC * TOKPC], F32, tag="tips")

            Hprev = None
            for layer in range(L):
                def loadp(dram_ap, shape, tag):
                    pt_ = par.tile(list(shape), F32R, tag=tag, name=f"par_{tag}")
                    nc.sync.dma_start(pt_[:], dram_ap)
                    return pt_

                ah = loadp(at_hi_d[layer, :, :], (128, 128), "ah")
                al = loadp(at_lo_d[layer, :, :], (128, 128), "al")
                bh = loadp(bt_hi_d[layer, :, :, :], (128, KC, 128), "bh")
                bl = loadp(bt_lo_d[layer, :, :, :], (128, KC, 128), "bl")
                ch = loadp(ct_hi_d[layer, :, :, :], (128, KC, 128), "ch")
                cl = loadp(ct_lo_d[layer, :, :, :], (128, KC, 128), "cl")
                dh = loadp(dd_hi_d[layer, :, :, :], (128, KC, 128), "dh")
                dl = loadp(dd_lo_d[layer, :, :, :], (128, KC, 128), "dl")

                def emit_mm2_lif2(t, H_t, xs_t, layer_):
                    # output update accumulation (v2, per chunk) + LIF2
                    for k in range(KC):
                        vsl = v2pr[k // 2][:, (k % 2) * TOKPC:(k % 2 + 1) * TOKPC]
                        mm2 = [(ch[:, k, :], H_t[:]), (cl[:, k, :], H_t[:]),
                               (dh[:, k, :], xs_t[k]), (dl[:, k, :], xs_t[k])]
                        for i, (lhsT, rhs) in enumerate(mm2):
                            # start=True clears the WHOLE bank -> only the
                            # first MM into each bank per layer may set it
                            nc.tensor.matmul(vsl, lhsT, rhs,
                                             start=(t == 0 and i == 0 and k % 2 == 0),
                                             stop=(i == len(mm2) - 1),
                                             skip_group_check=True)
                    # LIF2: sg on ACT (psum), mask on GPSIMD (sbuf),
                    # v2 *= mask on DVE (psum), spikes on DVE (one 1024 op,
                    # f32r out - the verifier requires DVE/GPSIMD producers
                    # for fp32r matmul operands)
                    m2c = lif.tile([128, 2 * 2 * TOKPC], F32, tag="m2c")
                    for j in range(2):
                        sg2 = lif.tile([128, 2 * TOKPC], F32, tag=f"sg2_{j}",
                                       name=f"sg2_{j}")
                        nc.scalar.activation(sg2[:], v2pr[j][:], ACTF.Sign,
                                             bias=neg2[:], scale=1.0)
                        nc.gpsimd.tensor_scalar(m2c[:, j * 512:(j + 1) * 512],
                                                sg2[:], -0.25, 0.25,
                                                OP.mult, OP.add)
                        nc.vector.tensor_tensor(v2pr[j][:], v2pr[j][:],
                                                m2c[:, j * 512:(j + 1) * 512],
                                                OP.mult)
                    xsl = xb[:, t * KC * 256:(t + 1) * KC * 256]
                    nc.vector.tensor_scalar(xsl, m2c[:], -2.0, 1.0,
                                            OP.mult, OP.add)
                    if layer_ == L - 1:
                        # time-integration on the PE: tips += I @ X[t]
                        # (reads xsl AFTER the spike overwrite above)
                        for k in range(KC):
                            nc.tensor.matmul(
                                tips[:, k * TOKPC:(k + 1) * TOKPC],
                                ident_r[:], xs_t[k],
                                start=(t == 0 and k % 2 == 0),
                                stop=(t == T - 1),
                                skip_group_check=True)

                prev = None  # (t, H, xs) pending MM2+LIF2 (1-step software skew)
                for t in range(T):
                    xs = [xb[:, (t * KC + k) * 256:(t * KC + k) * 256 + 256]
                          for k in range(KC)]
                    # ---- state update accumulation (v1) ----
                    mm1 = []
                    if t > 0:
                        mm1 += [(ah[:], Hprev[:]), (al[:], Hprev[:])]
                    for k in range(KC):
                        mm1 += [(bh[:, k, :], xs[k]), (bl[:, k, :], xs[k])]
                    for i, (lhsT, rhs) in enumerate(mm1):
                        nc.tensor.matmul(v1ps[:], lhsT, rhs,
                                         start=(t == 0 and i == 0),
                                         stop=(i == len(mm1) - 1),
                                         skip_group_check=True)
                    # ---- LIF1: spike H straight from PSUM on DVE (critical
                    #      path: f32r matmul operands need DVE/GPSIMD
                    #      producers), m1 derived off-path on GPSIMD ----
                    H = lif.tile([128, TOKPC], F32R, tag="H", bufs=3)
                    nc.vector.tensor_scalar(H[:], v1ps[:], 2.0, None, OP.is_ge)
                    m1 = lif.tile([128, TOKPC], F32, tag="m1")
                    nc.gpsimd.tensor_scalar(m1[:], H[:].bitcast(F32), -0.5, 0.5,
                                            OP.mult, OP.add)
                    nc.vector.tensor_tensor(v1ps[:], v1ps[:], m1[:], OP.mult)
                    # ---- previous step's output-side work (keeps PE fed) ----
                    if prev is not None:
                        emit_mm2_lif2(*prev, layer)
                    prev = (t, H, xs)
                    Hprev = H
                emit_mm2_lif2(*prev, layer)

            # time-integrated spike counts -> bf16 (exact integers 0..20;
            # the 1/T scale is folded into Wp on the host)
            for j in range(2):
                nc.scalar.activation(ti_bf[:, j * 512:(j + 1) * 512],
                                     tips[:, j * 512:(j + 1) * 512],
                                     ACTF.Copy, bias=0.0, scale=1.0)

        # ---------------- projection: own 256 tokens x full vocab ----------
        with tc.tile_pool(name="prjp", bufs=2, space="PSUM") as prjps:
            for g in range(NVG):
                wts = []
                for k in range(KC):
                    wt = prw.tile([128, NVW], BF16, tag=f"wt{k}", name=f"wt{k}")
                    nc.sync.dma_start(wt[:], wpt_d[k * 128:(k + 1) * 128,
                                                   g * NVW:(g + 1) * NVW])
                    wts.append(wt)
                for m in range(TOKPC // 128):
                    pos = [prjps.tile([128, NV], F32, tag=f"po{nv}",
                                      name=f"po{nv}") for nv in range(NVW // NV)]
                    for k in range(KC):
                        lh = ti_bf[:, k * 256 + m * 128: k * 256 + m * 128 + 128]
                        for nv in range(NVW // NV):
                            nc.tensor.matmul(pos[nv][:], lh,
                                             wts[k][:, nv * NV:(nv + 1) * NV],
                                             start=(k == 0), stop=(k == KC - 1),
                                             skip_group_check=True)
                    osb = osbp.tile([128, NVW], BF16, tag="osb")
                    for nv in range(NVW // NV):
                        nc.scalar.activation(osb[:, nv * NV:(nv + 1) * NV],
                                             pos[nv][:], ACTF.Copy,
                                             bias=0.0, scale=1.0)
                    nc.sync.dma_start(out_d[m * 128:(m + 1) * 128,
                                            g * NVW:(g + 1) * NVW], osb[:])

    nc.compile()
    return nc


_NC_CACHE = {}
_last_in_maps = None


def _get_nc():
    if "nc" not in _NC_CACHE:
        _NC_CACHE["nc"] = _build_nc()
    return _NC_CACHE["nc"]


def kernel(input_ids, emb_table, A, B, C, D, Wp, bp):
    input_ids = np.asarray(input_ids)
    emb_table = np.ascontiguousarray(np.asarray(emb_table), dtype=np.float32)
    A = np.asarray(A, dtype=np.float32)
    B = np.asarray(B, dtype=np.float32)
    C = np.asarray(C, dtype=np.float32)
    D = np.asarray(D, dtype=np.float32)
    Wp = np.asarray(Wp, dtype=np.float32)
    bp = np.asarray(bp, dtype=np.float32)

    ids_flat = input_ids.reshape(-1).astype(np.int32)          # (2048,)

    at = np.ascontiguousarray(A.transpose(0, 2, 1))            # (L,128,128)
    at_hi, at_lo = _hilo(at)
    Bh = B.copy()
    Bh[0] *= np.float32(0.5)   # layer-0 spikes arrive x2 scaled from encode
    bt = np.ascontiguousarray(
        Bh.transpose(2, 0, 1).reshape(KC, 128, L, DS).transpose(2, 1, 0, 3))
    # bt[l,p,k,m] = B[l, m, k*128+p]
    bt_hi, bt_lo = _hilo(bt)
    ct = np.ascontiguousarray(C.transpose(0, 2, 1).reshape(L, 128, KC, 128))
    # ct[l,p,mc,m] = C[l, mc*128+m, p]
    ct_hi, ct_lo = _hilo(ct)
    Dh = D.copy()
    Dh[0] *= np.float32(0.5)   # layer-0 spikes arrive x2 scaled from encode
    dc = Dh.reshape(L, KC, 128)                                # (L,KC,128)
    dc_hi, dc_lo = _hilo(dc)
    eye = np.eye(128, dtype=np.float32)
    # diag matrices for the D (elementwise) term, built on host
    dd_hi = np.ascontiguousarray(
        (eye[None, None, :, :] * dc_hi[:, :, None, :]).transpose(0, 2, 1, 3))
    dd_lo = np.ascontiguousarray(
        (eye[None, None, :, :] * dc_lo[:, :, None, :]).transpose(0, 2, 1, 3))
    # dd[l, p, k, m] = D[l, k*128+p] if p == m else 0

    wpt = np.ascontiguousarray(Wp.T) * np.float32(1.0 / T)     # (512, 32000) f32
    wpt_bf = wpt.astype(ml_dtypes.bfloat16)

    nc = _get_nc()
    in_maps = []
    for c in range(NCORES):
        ids_c = ids_flat[c * TOKPC:(c + 1) * TOKPC].reshape(2, 128, 1)
        in_maps.append({
            "ids": np.ascontiguousarray(ids_c),
            "emb": emb_table,
            "at_hi": at_hi, "at_lo": at_lo,
            "bt_hi": bt_hi, "bt_lo": bt_lo,
            "ct_hi": ct_hi, "ct_lo": ct_lo,
            "dd_hi": dd_hi, "dd_lo": dd_lo,
            "wpt": wpt_bf,
        })

    global _last_in_maps
    _last_in_maps = in_maps
    res = run_bass_kernel_spmd(nc, in_maps, core_ids=list(range(NCORES)))
    outs = [res.results[c]["out"].astype(np.float32) for c in range(NCORES)]
    full = np.concatenate(outs, axis=0)                        # (2048, 32000)
    full += bp[None, :]
    return full.reshape(BATCH, SEQ, VOC).astype(np.float32)
